# revision 36
# baseline (speedup 1.0000x reference)
"""4-layer tanh RNN on 8 Trainium2 NeuronCores.

Strategy: 4-stage layer pipeline x 2-way batch split. Core c handles
layer c//2 for batch half c%2. Time is processed in blocks of T=32
steps; each round every core gathers its input block (previous stage's
output), projects it (x @ WxT + b), runs 32 recurrence steps
(weight-stationary bf16 matmuls, zT[d_out,b] layout so h never needs a
transpose), then hands its output block to the next stage.

Pipelining (v4): the handoff is split into half-blocks. The first half
(t<16) ships mid-round, so its AllGather and the successor's gather +
projection all overlap the producer's second half; the second half
ships at round end and is projected by the consumer mid-next-round.
Projection matmuls are interleaved a few per recurrence step, riding
the PE idle gaps the per-step tanh dependency creates, so projection
costs almost no wall-clock. Stage handoff uses one 4-core-group
AllGather per half along each batch-half chain (group position = stage,
2x0.5MB out/core/round vs 8MB for a flat 8-way gather); stage-0 cores
read their x-feed from a board region staged by local DMA. One
SPMD-uniform indirect gather with a per-core constant row index reads
the board.

Recurrence step: two PSUM groups (m-halves) in bank-padded tiles; an
identity matmul seeds xw into each group (start=True clears the bank),
32 wh matmuls accumulate, tanh reads PSUM directly as soon as its
half's group stops.

Compute dtype bf16 (PE fp32 is 4x slower), fp32 PSUM accumulation, fp32
tanh. xw is stored bf16 (it feeds the identity matmul).
"""
import sys
import numpy as np

if "/opt/trn_rl_repo" not in sys.path:
    sys.path.insert(0, "/opt/trn_rl_repo")

import ml_dtypes

BF = ml_dtypes.bfloat16

# Problem config (hardcoded per contract)
import os as _os
B, L, D, NL = 16, 512, 1024, 4
P = 128
KT = D // P          # 8 k-tiles (contraction)
MT = D // P          # 8 m-tiles (output)
BC = B // 2          # 8 = per-core batch half
T = 32               # timesteps per block
TH = T // 2          # 16 = first-chunk timesteps
TQ = T // 4          # 8 = late-chunk timesteps
NB = L // T          # 16 blocks
ROUNDS = NB + NL - 1  # 19
N_CORES = 8
BLK_COLS = MT * T * BC   # 2048 block columns
HALF_COLS = MT * TH * BC  # 1024 first-chunk columns: col = m*TH*BC + tl*BC + b
QCOLS = MT * TQ * BC      # 512 late-chunk columns
# handoff chunks: (t-range start, t-range len, column count)
CHUNKS = [(0, TH, HALF_COLS), (TH, TQ, QCOLS), (TH + TQ, TQ, QCOLS)]

# one chain per batch half: stage s core is group position s, so each
# receiver reads its predecessor's block at rows (s-1)*P of the AG out.
GROUPS = [[0, 2, 4, 6], [1, 3, 5, 7]]

_cache = {}


def _build():
    import concourse.bass as bass
    import concourse.mybir as mybir
    import concourse.tile as tile
    from concourse import bacc
    from concourse.tile import add_dep_helper

    F32 = mybir.dt.float32
    BF16 = mybir.dt.bfloat16
    I32 = mybir.dt.int32
    Tanh = mybir.ActivationFunctionType.Tanh

    nc = bacc.Bacc("TRN2", target_bir_lowering=False, debug=False,
                   num_devices=N_CORES)

    # ---- I/O ----
    whT = nc.dram_tensor("whT", [P, KT * MT * P], BF16, kind="ExternalInput")
    wxT = nc.dram_tensor("wxT", [P, KT * MT * P], BF16, kind="ExternalInput")
    bias = nc.dram_tensor("bias", [P, MT], F32, kind="ExternalInput")
    carry = nc.dram_tensor("carry", [ROUNDS, P, KT * BC], mybir.dt.uint8, kind="ExternalInput")
    cinit = nc.dram_tensor("cinit", [ROUNDS, P, KT * BC], BF16, kind="ExternalInput")
    gidx = nc.dram_tensor("gidx", [P, 1], I32, kind="ExternalInput")
    ident = nc.dram_tensor("ident", [P, P], BF16, kind="ExternalInput")
    # x-feed, halved column layout: col = h*HALF_COLS + k*TH*BC + tl*BC + b
    x0t = nc.dram_tensor("x0t", [ROUNDS, P, BLK_COLS], BF16, kind="ExternalInput")
    out = nc.dram_tensor("out", [ROUNDS, P, BLK_COLS], F32, kind="ExternalOutput")

    # handoff boards, one per half-block per round: rows [0:4P) AllGather
    # out (one row-block per chain stage), [4P:5P) x-feed staged by DMA.
    hands = [[nc.dram_tensor(f"hand{h}_{r}", [5 * P, CHUNKS[h][2]], BF16)
              for r in range(ROUNDS)] for h in range(3)]
    hins = [[nc.dram_tensor(f"hin{h}_{r}", [P, CHUNKS[h][2]], BF16)
             for r in range(ROUNDS - 1)] for h in range(3)]
    wu_in = nc.dram_tensor("wu_in", [P, 64], BF16)
    wu_out = nc.dram_tensor("wu_out", [4 * P, 64], BF16)

    with tile.TileContext(nc) as tc:
        with (
            tc.tile_pool(name="const", bufs=1) as cpool,
            tc.tile_pool(name="xw", bufs=1) as xwpool,
            tc.tile_pool(name="blk", bufs=1) as blkpool,
            tc.tile_pool(name="hs", bufs=2) as hspool,
            tc.tile_pool(name="o32", bufs=2) as opool,
            tc.tile_pool(name="psr", bufs=2, space="PSUM") as prpool,
            tc.tile_pool(name="psp", bufs=2, space="PSUM") as pppool,
        ):
            wh_sb = cpool.tile([P, KT, MT, P], BF16, tag="wh")
            nc.sync.dma_start(wh_sb[:], whT.ap().rearrange("p (k m q) -> p k m q", k=KT, m=MT))
            wx_sb = cpool.tile([P, KT, MT, P], BF16, tag="wx")
            nc.sync.dma_start(wx_sb[:], wxT.ap().rearrange("p (k m q) -> p k m q", k=KT, m=MT))
            bias_sb = cpool.tile([P, MT], F32, tag="bias")
            nc.sync.dma_start(bias_sb[:], bias[:])
            carry_sb = cpool.tile([P, ROUNDS, KT * BC], mybir.dt.uint8, tag="carry")
            nc.sync.dma_start(carry_sb[:], carry.ap().rearrange("r p c -> p r c"))
            cinit_sb = cpool.tile([P, ROUNDS, KT * BC], BF16, tag="cinit")
            nc.sync.dma_start(cinit_sb[:], cinit.ap().rearrange("r p c -> p r c"))
            gidx_sb = cpool.tile([P, 1], I32, tag="gidx")
            nc.sync.dma_start(gidx_sb[:], gidx[:])
            id_sb = cpool.tile([P, P], BF16, tag="ident")
            nc.sync.dma_start(id_sb[:], ident[:])

            # persistent double buffers, alternated by round parity
            blkA = blkpool.tile([P, MT, T, BC], BF16, tag="blkA")
            blkB = blkpool.tile([P, MT, T, BC], BF16, tag="blkB")
            nc.vector.memset(blkA[:], 0.0)
            nc.vector.memset(blkB[:], 0.0)
            # gathered input block, chunk-major cols (h)(k, tl, b)
            xbA = blkpool.tile([P, BLK_COLS], BF16, tag="xbA")
            xbB = blkpool.tile([P, BLK_COLS], BF16, tag="xbB")
            CH_OFF = [0, HALF_COLS, HALF_COLS + QCOLS]

            xw_sb = xwpool.tile([P, MT, T, BC], BF16, tag="xw")

            def gather(xb, h, r, deps):
                g = nc.gpsimd.indirect_dma_start(
                    out=xb[:, CH_OFF[h]:CH_OFF[h] + CHUNKS[h][2]],
                    out_offset=None,
                    in_=hands[h][r][:],
                    in_offset=bass.IndirectOffsetOnAxis(ap=gidx_sb[:, :1], axis=0),
                )
                for dep, reason in deps:
                    add_dep_helper(g.ins, dep.ins, sync=True, reason=reason)
                return g

            def proj_chunk(pp, xb, h, mp, ks, ke, after=None):
                """k-range slice of an m-pair projection group for
                t-chunk h.

                Matmuls are chained sequentially and anchored after
                `after` (sync=False scheduling edges) so the scheduler
                cannot float them to the head of the PE queue, where
                their gather dependency would head-of-line-block the
                recurrence.
                """
                t0, tn, _ = CHUNKS[h]
                prev_mm = None
                for k in range(ks, ke):
                    for mi in range(2):
                        m = 2 * mp + mi
                        mm = nc.tensor.matmul(
                            pp[:, mi, 0:tn, 0:BC],
                            wx_sb[:, k, m, :],
                            xb[:, CH_OFF[h] + k * tn * BC:CH_OFF[h] + (k + 1) * tn * BC],
                            start=(k == 0 and mi == 0),
                            stop=(k == KT - 1 and mi == 1),
                            skip_group_check=True,
                        )
                        if prev_mm is None:
                            if after is not None:
                                add_dep_helper(mm.ins, after.ins, sync=False,
                                               reason="hold proj in place")
                        else:
                            add_dep_helper(mm.ins, prev_mm.ins, sync=False,
                                           reason="keep chunk contiguous")
                        prev_mm = mm
                if ke == KT:
                    nc.vector.tensor_tensor(
                        out=xw_sb[:, 2 * mp:2 * mp + 2, t0:t0 + tn, :],
                        in0=pp[:, :, 0:tn, 0:BC],
                        in1=bias_sb[:, 2 * mp:2 * mp + 2, None, None].to_broadcast((P, 2, tn, BC)),
                        op=mybir.AluOpType.add,
                    )

            def proj_pair(xb, h, mp, after=None):
                pp = pppool.tile([P, 2, TH, 16], mybir.dt.float32, tag="pp")
                proj_chunk(pp, xb, h, mp, 0, 4, after)
                proj_chunk(pp, xb, h, mp, 4, 8)

            def rec_step(cur, hstart, t):
                """Returns the step's last wh matmul (proj anchor)."""
                pslo = prpool.tile([P, 4, 128], mybir.dt.float32, tag="pslo")
                pshi = prpool.tile([P, 4, 128], mybir.dt.float32, tag="pshi")
                idlo = nc.tensor.matmul(
                    pslo[:, :, 0:BC], id_sb[:], xw_sb[:, 0:4, t, :],
                    start=True, stop=False, skip_group_check=True,
                )
                idhi = nc.tensor.matmul(
                    pshi[:, :, 0:BC], id_sb[:], xw_sb[:, 4:8, t, :],
                    start=True, stop=False, skip_group_check=True,
                )
                for half in range(2):
                    ps = pslo if half == 0 else pshi
                    idm = idlo if half == 0 else idhi
                    for k in range(KT):
                        if t == 0:
                            rhs = hstart[:, k * BC:(k + 1) * BC]
                        else:
                            rhs = cur[:, k, t - 1, :]
                        for mi in range(4):
                            m = 4 * half + mi
                            mm = nc.tensor.matmul(
                                ps[:, mi, 0:BC],
                                wh_sb[:, k, m, :],
                                rhs,
                                start=False,
                                stop=(k == KT - 1 and mi == 3),
                                skip_group_check=True,
                            )
                            if k == 0:
                                add_dep_helper(mm.ins, idm.ins, sync=False,
                                               reason="bank clear first")
                    nc.scalar.activation(
                        cur[:, 4 * half:4 * half + 4, t, :],
                        ps[:, :, 0:BC], Tanh
                    )
                return mm

            # warm up the collective ring before the first real handoff
            # (the first AllGather pays ~25us of one-time setup)
            nc.gpsimd.collective_compute(
                "AllGather",
                mybir.AluOpType.bypass,
                replica_groups=GROUPS,
                ins=[wu_in[:]],
                outs=[wu_out[:]],
            )

            # stage round-0 x-feeds
            dx_prev = [
                nc.sync.dma_start(hands[h][0][4 * P:5 * P, :],
                                  x0t[0][:, CH_OFF[h]:CH_OFF[h] + CHUNKS[h][2]])
                for h in range(3)
            ]

            ccs = [None, None, None]
            for r in range(ROUNDS):
                cur = blkA if r % 2 == 0 else blkB
                prev = blkB if r % 2 == 0 else blkA
                xb = xbA if r % 2 == 0 else xbB
                xb_next = xbB if r % 2 == 0 else xbA

                if r == 0:
                    # no prior round overlapped this work: gather+project
                    # everything up front.
                    for h in range(3):
                        gather(xb, h, 0, [(dx_prev[h], "gather after x-feed")])
                    for h in range(3):
                        for mp in range(4):
                            proj_pair(xb, h, mp)
                    g_c1 = g_c2 = None
                else:
                    # late-chunk gathers for this round; their AGs
                    # launched at steps 23 / 31 of round r-1, so these
                    # waits resolve early in this round.  Projections are
                    # interleaved into steps 0..7 / 8..15 below.
                    g_c1 = gather(xb, 1, r, [(dx_prev[1], "gather after x-feed"),
                                             (ccs[1], "gather after AG")])
                    g_c2 = gather(xb, 2, r, [(dx_prev[2], "gather after x-feed"),
                                             (ccs[2], "gather after AG")])
                    # g_c2's wait resolves ~4 steps into the round; if
                    # the scheduler put it before g_c1 on the gpsimd
                    # queue, g_c1 (and the steps-0..7 projections) would
                    # block behind it.
                    add_dep_helper(g_c2.ins, g_c1.ins, sync=False,
                                   reason="queue order")

                # h_start = carry ? prev_block_tail : cinit
                hstart = hspool.tile([P, KT * BC], BF16, tag="hs")
                nc.vector.tensor_copy(hstart[:], cinit_sb[:, r])
                nc.vector.copy_predicated(
                    hstart[:], carry_sb[:, r], prev[:, :, T - 1, :]
                )

                def ship(h, t_lo, t_n):
                    """Send chunk h of cur to the successor's board."""
                    dd = nc.sync.dma_start(
                        hins[h][r].ap().rearrange("p (m t b) -> p m t b",
                                                  m=MT, t=t_n),
                        cur[:, :, t_lo:t_lo + t_n, :],
                    )
                    dxn = nc.sync.dma_start(
                        hands[h][r + 1][4 * P:5 * P, :],
                        x0t[r + 1][:, CH_OFF[h]:CH_OFF[h] + CHUNKS[h][2]])
                    cc = nc.gpsimd.collective_compute(
                        "AllGather",
                        mybir.AluOpType.bypass,
                        replica_groups=GROUPS,
                        ins=[hins[h][r][:]],
                        outs=[hands[h][r + 1][0:4 * P, :]],
                    )
                    add_dep_helper(cc.ins, dd.ins, sync=True,
                                   reason="AG after blk dma")
                    # keep the gpsimd queue in program order: an AG
                    # trigger floated ahead of a pending gather would
                    # head-of-line-block it on this trigger's DMA wait.
                    if g_c2 is not None:
                        add_dep_helper(cc.ins, g_c2.ins, sync=False,
                                       reason="queue order")
                    ccs[h] = cc
                    dx_prev[h] = dxn
                    return cc

                pp_live = None
                g_c0n = None
                for t in range(T):
                    last_mm = rec_step(cur, hstart, t)

                    # interleave projections into the tanh-latency gaps,
                    # half an m-pair k-sweep per step: steps 0..7 project
                    # this round's chunk 1 (t16..23), steps 8..15 chunk 2
                    # (t24..31), steps 24..31 the next round's chunk 0
                    # (gathered below after its mid-round AG).
                    if t < 8 and r > 0:
                        mp, kr = divmod(t, 2)
                        if kr == 0:
                            pp_live = pppool.tile([P, 2, TH, 16],
                                                  mybir.dt.float32, tag="pp")
                        proj_chunk(pp_live, xb, 1, mp, 4 * kr, 4 * kr + 4,
                                   after=last_mm)
                    if 8 <= t < 16 and r > 0:
                        mp, kr = divmod(t - 8, 2)
                        if kr == 0:
                            pp_live = pppool.tile([P, 2, TH, 16],
                                                  mybir.dt.float32, tag="pp")
                        proj_chunk(pp_live, xb, 2, mp, 4 * kr, 4 * kr + 4,
                                   after=last_mm)
                    if 24 <= t < 32 and r < ROUNDS - 1:
                        mp, kr = divmod(t - 24, 2)
                        if kr == 0:
                            pp_live = pppool.tile([P, 2, TH, 16],
                                                  mybir.dt.float32, tag="pp")
                        proj_chunk(pp_live, xb_next, 0, mp, 4 * kr, 4 * kr + 4,
                                   after=last_mm)

                    if r < ROUNDS - 1:
                        if t == TH - 1:
                            cc0 = ship(0, 0, TH)
                            g_c0n = gather(xb_next, 0, r + 1,
                                           [(cc0, "gather after AG"),
                                            (dx_prev[0], "gather after x-feed")])
                        elif t == TH + TQ - 1:
                            cc1 = ship(1, TH, TQ)
                            if g_c0n is not None:
                                add_dep_helper(cc1.ins, g_c0n.ins, sync=False,
                                               reason="queue order")

                # ship the last chunk at round end, BEFORE the fp32
                # output copy: the shared Sync DMA queue would otherwise
                # delay the AG launch behind the 1.2us cast.
                if r < ROUNDS - 1:
                    cc2 = ship(2, TH + TQ, TQ)
                    if g_c0n is not None:
                        add_dep_helper(cc2.ins, g_c0n.ins, sync=False,
                                       reason="queue order")

                # write fp32 output block
                o32 = opool.tile([P, MT * T * BC], F32, tag="o32")
                nc.vector.tensor_copy(o32[:], cur[:])
                nc.sync.dma_start(out[r], o32[:])
    nc.compile()
    return nc


def _prep_inputs(X, h0s, W, b):
    """Build the 8 per-core input maps."""
    in_maps = []
    for c in range(N_CORES):
        s, j = c // 2, c % 2
        Wl = np.asarray(W[s], dtype=np.float32)
        Wx, Wh = Wl[:, :D], Wl[:, D:]

        def tiles(M):  # M: [e, d] -> lhsT tiles [p, (k, m, q)]
            A = M.reshape(MT, P, KT, P)          # [m, q, k, p]
            return np.ascontiguousarray(
                A.transpose(3, 2, 0, 1).reshape(P, KT * MT * P)).astype(BF)

        whT = tiles(Wh)
        wxT = tiles(Wx)
        bias = np.ascontiguousarray(
            np.asarray(b[s], np.float32).reshape(MT, P).T)

        hin = np.asarray(h0s[s, BC * j:BC * (j + 1)], np.float32)  # [b, d]
        hinit = np.ascontiguousarray(
            hin.reshape(BC, KT, P).transpose(2, 1, 0).reshape(P, KT * BC)).astype(BF)

        carry = np.zeros((ROUNDS, P, KT * BC), np.uint8)
        cinit = np.zeros((ROUNDS, P, KT * BC), BF)
        for r in range(ROUNDS):
            if r > s:
                carry[r] = 1
            else:
                cinit[r] = hinit

        x0t = np.zeros((ROUNDS, P, BLK_COLS), BF)
        if s == 0:
            Xj = np.asarray(X[BC * j:BC * (j + 1)], np.float32)  # [b, L, d]
            # chunk-major: [(k,16t,b); (k,8t,b); (k,8t,b)]
            Xr = Xj.reshape(BC, NB, T, KT, P)  # [b, blk, t, k, p]
            parts = []
            for t_lo, t_n, _ in ((0, TH, 0), (TH, TQ, 0), (TH + TQ, TQ, 0)):
                c = Xr[:, :, t_lo:t_lo + t_n]  # [b, blk, tn, k, p]
                c = c.transpose(1, 4, 3, 2, 0)  # [blk, p, k, tn, b]
                parts.append(c.reshape(NB, P, KT * t_n * BC))
            Xb = np.ascontiguousarray(np.concatenate(parts, axis=2)).astype(BF)
            x0t[0:NB] = Xb
            gidx = (4 * P + np.arange(P, dtype=np.int32)).reshape(P, 1)
        else:  # stage s reads its predecessor (group position s-1)
            gidx = ((s - 1) * P + np.arange(P, dtype=np.int32)).reshape(P, 1)

        in_maps.append({
            "whT": whT, "wxT": wxT, "bias": bias,
            "carry": carry, "cinit": cinit,
            "gidx": gidx, "ident": np.eye(P, dtype=BF),
            "x0t": x0t,
        })
    return in_maps


def _extract(results):
    """Assemble full output [B, L, D] from stage-3 cores (6, 7)."""
    Y = np.empty((B, L, D), np.float32)
    for j in range(2):
        o = results[6 + j]["out"][NL - 1:NL - 1 + NB]   # [q, p, cols]
        o = o.reshape(NB, P, MT, T, BC).transpose(4, 0, 3, 2, 1)  # [b,q,t,m,p]
        Y[BC * j:BC * (j + 1)] = o.reshape(BC, L, D)
    return Y


def kernel(X, h0s, W, b, _trace=False):
    from concourse.bass_utils import run_bass_kernel_spmd

    if "nc" not in _cache:
        _cache["nc"] = _build()
    nc = _cache["nc"]
    in_maps = _prep_inputs(np.asarray(X), np.asarray(h0s), np.asarray(W),
                           np.asarray(b))
    res = run_bass_kernel_spmd(nc, in_maps, core_ids=list(range(N_CORES)),
                               trace=_trace)
    _cache["last_results"] = res
    return _extract(res.results)


# revision 37
# speedup vs baseline: 1.0118x; 1.0118x over previous
"""4-layer tanh RNN on 8 Trainium2 NeuronCores.

Strategy: 4-stage layer pipeline x 2-way batch split. Core c handles
layer c//2 for batch half c%2. Time is processed in blocks of T=32
steps; each round every core gathers its input block (previous stage's
output), projects it (x @ WxT + b), runs 32 recurrence steps
(weight-stationary bf16 matmuls, zT[d_out,b] layout so h never needs a
transpose), then hands its output block to the next stage.

Pipelining: the handoff is split into three chunks (t 0..15, 16..23,
24..31) shipped at steps 15 / 23 / 31, so each chunk's AllGather and
the successor's gather + projection overlap the producer's remaining
steps, and every chunk's projection can be interleaved into the
consumer's own recurrence (steps 0..7 / 8..15 / 24..31, half an m-pair
k-sweep per step) riding the PE idle gap the per-step tanh dependency
creates. Stage handoff uses 4-core-group AllGathers along each
batch-half chain (group position = stage; ~2MB out/core/round total vs
8MB for a flat 8-way gather); stage-0 cores read their x-feed from a
board region staged by local DMA. One SPMD-uniform indirect gather with
a per-core constant row index reads each board. A tiny warmup AllGather
absorbs the ~25us first-collective setup. Explicit sync=False
scheduling edges pin the projection matmuls and the gpsimd queue order
in place -- the Tile scheduler otherwise floats work to where a pending
collective dependency head-of-line-blocks an engine queue.

Recurrence step: two PSUM groups (m-halves) in bank-padded tiles (one
group per 2KB bank -- start=True clears the whole bank); an identity
matmul seeds xw into each group, 32 wh matmuls accumulate, tanh reads
PSUM directly as soon as its half's group stops, so the first half's
tanh overlaps the second half's matmuls.

Compute dtype bf16 (PE fp32 is 4x slower), fp32 PSUM accumulation, fp32
tanh. xw is stored bf16 (it feeds the identity matmul).

Measured on the shared axon trn2 (NTFF-profiled): 1.63ms vs 2.69ms for
the original single 8-way-AllGather version, rel err 1.17e-2.
"""
import sys
import numpy as np

if "/opt/trn_rl_repo" not in sys.path:
    sys.path.insert(0, "/opt/trn_rl_repo")

import ml_dtypes

BF = ml_dtypes.bfloat16

# Problem config (hardcoded per contract)
import os as _os
B, L, D, NL = 16, 512, 1024, 4
P = 128
KT = D // P          # 8 k-tiles (contraction)
MT = D // P          # 8 m-tiles (output)
BC = B // 2          # 8 = per-core batch half
T = 32               # timesteps per block
TH = T // 2          # 16 = first-chunk timesteps
TQ = T // 4          # 8 = late-chunk timesteps
NB = L // T          # 16 blocks
ROUNDS = NB + NL - 1  # 19
N_CORES = 8
BLK_COLS = MT * T * BC   # 2048 block columns
HALF_COLS = MT * TH * BC  # 1024 first-chunk columns: col = m*TH*BC + tl*BC + b
QCOLS = MT * TQ * BC      # 512 late-chunk columns
# handoff chunks: (t-range start, t-range len, column count)
CHUNKS = [(0, TH, HALF_COLS), (TH, TQ, QCOLS), (TH + TQ, TQ, QCOLS)]

# one chain per batch half: stage s core is group position s, so each
# receiver reads its predecessor's block at rows (s-1)*P of the AG out.
GROUPS = [[0, 2, 4, 6], [1, 3, 5, 7]]

_cache = {}


def _build():
    import concourse.bass as bass
    import concourse.mybir as mybir
    import concourse.tile as tile
    from concourse import bacc
    from concourse.tile import add_dep_helper

    F32 = mybir.dt.float32
    BF16 = mybir.dt.bfloat16
    I32 = mybir.dt.int32
    Tanh = mybir.ActivationFunctionType.Tanh

    nc = bacc.Bacc("TRN2", target_bir_lowering=False, debug=False,
                   num_devices=N_CORES)

    # ---- I/O ----
    whT = nc.dram_tensor("whT", [P, KT * MT * P], BF16, kind="ExternalInput")
    wxT = nc.dram_tensor("wxT", [P, KT * MT * P], BF16, kind="ExternalInput")
    bias = nc.dram_tensor("bias", [P, MT], F32, kind="ExternalInput")
    carry = nc.dram_tensor("carry", [ROUNDS, P, KT * BC], mybir.dt.uint8, kind="ExternalInput")
    cinit = nc.dram_tensor("cinit", [ROUNDS, P, KT * BC], BF16, kind="ExternalInput")
    gidx = nc.dram_tensor("gidx", [P, 1], I32, kind="ExternalInput")
    ident = nc.dram_tensor("ident", [P, P], BF16, kind="ExternalInput")
    # x-feed, halved column layout: col = h*HALF_COLS + k*TH*BC + tl*BC + b
    x0t = nc.dram_tensor("x0t", [ROUNDS, P, BLK_COLS], BF16, kind="ExternalInput")
    out = nc.dram_tensor("out", [ROUNDS, P, BLK_COLS], F32, kind="ExternalOutput")

    # handoff boards, one per half-block per round: rows [0:4P) AllGather
    # out (one row-block per chain stage), [4P:5P) x-feed staged by DMA.
    hands = [[nc.dram_tensor(f"hand{h}_{r}", [5 * P, CHUNKS[h][2]], BF16)
              for r in range(ROUNDS)] for h in range(3)]
    hins = [[nc.dram_tensor(f"hin{h}_{r}", [P, CHUNKS[h][2]], BF16)
             for r in range(ROUNDS - 1)] for h in range(3)]
    wu_in = nc.dram_tensor("wu_in", [P, 64], BF16)
    wu_out = nc.dram_tensor("wu_out", [4 * P, 64], BF16)

    with tile.TileContext(nc) as tc:
        with (
            tc.tile_pool(name="const", bufs=1) as cpool,
            tc.tile_pool(name="xw", bufs=1) as xwpool,
            tc.tile_pool(name="blk", bufs=1) as blkpool,
            tc.tile_pool(name="hs", bufs=2) as hspool,
            tc.tile_pool(name="o32", bufs=2) as opool,
            tc.tile_pool(name="psr", bufs=2, space="PSUM") as prpool,
            tc.tile_pool(name="psp", bufs=2, space="PSUM") as pppool,
        ):
            wh_sb = cpool.tile([P, KT, MT, P], BF16, tag="wh")
            nc.sync.dma_start(wh_sb[:], whT.ap().rearrange("p (k m q) -> p k m q", k=KT, m=MT))
            wx_sb = cpool.tile([P, KT, MT, P], BF16, tag="wx")
            nc.sync.dma_start(wx_sb[:], wxT.ap().rearrange("p (k m q) -> p k m q", k=KT, m=MT))
            bias_sb = cpool.tile([P, MT], F32, tag="bias")
            nc.sync.dma_start(bias_sb[:], bias[:])
            carry_sb = cpool.tile([P, ROUNDS, KT * BC], mybir.dt.uint8, tag="carry")
            nc.sync.dma_start(carry_sb[:], carry.ap().rearrange("r p c -> p r c"))
            cinit_sb = cpool.tile([P, ROUNDS, KT * BC], BF16, tag="cinit")
            nc.sync.dma_start(cinit_sb[:], cinit.ap().rearrange("r p c -> p r c"))
            gidx_sb = cpool.tile([P, 1], I32, tag="gidx")
            nc.sync.dma_start(gidx_sb[:], gidx[:])
            id_sb = cpool.tile([P, P], BF16, tag="ident")
            nc.sync.dma_start(id_sb[:], ident[:])

            # persistent double buffers, alternated by round parity
            blkA = blkpool.tile([P, MT, T, BC], BF16, tag="blkA")
            blkB = blkpool.tile([P, MT, T, BC], BF16, tag="blkB")
            nc.vector.memset(blkA[:], 0.0)
            nc.vector.memset(blkB[:], 0.0)
            # gathered input block, chunk-major cols (h)(k, tl, b)
            xbA = blkpool.tile([P, BLK_COLS], BF16, tag="xbA")
            xbB = blkpool.tile([P, BLK_COLS], BF16, tag="xbB")
            CH_OFF = [0, HALF_COLS, HALF_COLS + QCOLS]

            xw_sb = xwpool.tile([P, MT, T, BC], BF16, tag="xw")

            def gather(xb, h, r, deps):
                g = nc.gpsimd.indirect_dma_start(
                    out=xb[:, CH_OFF[h]:CH_OFF[h] + CHUNKS[h][2]],
                    out_offset=None,
                    in_=hands[h][r][:],
                    in_offset=bass.IndirectOffsetOnAxis(ap=gidx_sb[:, :1], axis=0),
                )
                for dep, reason in deps:
                    add_dep_helper(g.ins, dep.ins, sync=True, reason=reason)
                return g

            def proj_chunk(pp, xb, h, mp, ks, ke, after=None):
                """k-range slice of an m-pair projection group for
                t-chunk h.

                Matmuls are chained sequentially and anchored after
                `after` (sync=False scheduling edges) so the scheduler
                cannot float them to the head of the PE queue, where
                their gather dependency would head-of-line-block the
                recurrence.
                """
                t0, tn, _ = CHUNKS[h]
                prev_mm = None
                for k in range(ks, ke):
                    for mi in range(2):
                        m = 2 * mp + mi
                        mm = nc.tensor.matmul(
                            pp[:, mi, 0:tn, 0:BC],
                            wx_sb[:, k, m, :],
                            xb[:, CH_OFF[h] + k * tn * BC:CH_OFF[h] + (k + 1) * tn * BC],
                            start=(k == 0 and mi == 0),
                            stop=(k == KT - 1 and mi == 1),
                            skip_group_check=True,
                        )
                        if prev_mm is None:
                            if after is not None:
                                add_dep_helper(mm.ins, after.ins, sync=False,
                                               reason="hold proj in place")
                        else:
                            add_dep_helper(mm.ins, prev_mm.ins, sync=False,
                                           reason="keep chunk contiguous")
                        prev_mm = mm
                if ke == KT:
                    nc.vector.tensor_tensor(
                        out=xw_sb[:, 2 * mp:2 * mp + 2, t0:t0 + tn, :],
                        in0=pp[:, :, 0:tn, 0:BC],
                        in1=bias_sb[:, 2 * mp:2 * mp + 2, None, None].to_broadcast((P, 2, tn, BC)),
                        op=mybir.AluOpType.add,
                    )

            def proj_pair(xb, h, mp, after=None):
                pp = pppool.tile([P, 2, TH, 16], mybir.dt.float32, tag="pp")
                proj_chunk(pp, xb, h, mp, 0, 4, after)
                proj_chunk(pp, xb, h, mp, 4, 8)

            def rec_step(cur, hstart, t):
                """Returns the step's last wh matmul (proj anchor)."""
                pslo = prpool.tile([P, 4, 128], mybir.dt.float32, tag="pslo")
                pshi = prpool.tile([P, 4, 128], mybir.dt.float32, tag="pshi")
                idlo = nc.tensor.matmul(
                    pslo[:, :, 0:BC], id_sb[:], xw_sb[:, 0:4, t, :],
                    start=True, stop=False, skip_group_check=True,
                )
                idhi = nc.tensor.matmul(
                    pshi[:, :, 0:BC], id_sb[:], xw_sb[:, 4:8, t, :],
                    start=True, stop=False, skip_group_check=True,
                )
                for half in range(2):
                    ps = pslo if half == 0 else pshi
                    idm = idlo if half == 0 else idhi
                    for k in range(KT):
                        if t == 0:
                            rhs = hstart[:, k * BC:(k + 1) * BC]
                        else:
                            rhs = cur[:, k, t - 1, :]
                        for mi in range(4):
                            m = 4 * half + mi
                            mm = nc.tensor.matmul(
                                ps[:, mi, 0:BC],
                                wh_sb[:, k, m, :],
                                rhs,
                                start=False,
                                stop=(k == KT - 1 and mi == 3),
                                skip_group_check=True,
                            )
                            if k == 0:
                                add_dep_helper(mm.ins, idm.ins, sync=False,
                                               reason="bank clear first")
                    nc.scalar.activation(
                        cur[:, 4 * half:4 * half + 4, t, :],
                        ps[:, :, 0:BC], Tanh
                    )
                return mm

            # warm up the collective ring before the first real handoff
            # (the first AllGather pays ~25us of one-time setup)
            nc.gpsimd.collective_compute(
                "AllGather",
                mybir.AluOpType.bypass,
                replica_groups=GROUPS,
                ins=[wu_in[:]],
                outs=[wu_out[:]],
            )

            # stage round-0 x-feeds
            dx_prev = [
                nc.sync.dma_start(hands[h][0][4 * P:5 * P, :],
                                  x0t[0][:, CH_OFF[h]:CH_OFF[h] + CHUNKS[h][2]])
                for h in range(3)
            ]

            ccs = [None, None, None]
            for r in range(ROUNDS):
                cur = blkA if r % 2 == 0 else blkB
                prev = blkB if r % 2 == 0 else blkA
                xb = xbA if r % 2 == 0 else xbB
                xb_next = xbB if r % 2 == 0 else xbA

                if r == 0:
                    # no prior round overlapped this work: gather+project
                    # everything up front.
                    for h in range(3):
                        gather(xb, h, 0, [(dx_prev[h], "gather after x-feed")])
                    for h in range(3):
                        for mp in range(4):
                            proj_pair(xb, h, mp)
                    g_c1 = g_c2 = None
                else:
                    # late-chunk gathers for this round; their AGs
                    # launched at steps 23 / 31 of round r-1, so these
                    # waits resolve early in this round.  Projections are
                    # interleaved into steps 0..7 / 8..15 below.
                    g_c1 = gather(xb, 1, r, [(dx_prev[1], "gather after x-feed"),
                                             (ccs[1], "gather after AG")])
                    g_c2 = gather(xb, 2, r, [(dx_prev[2], "gather after x-feed"),
                                             (ccs[2], "gather after AG")])
                    # g_c2's wait resolves ~4 steps into the round; if
                    # the scheduler put it before g_c1 on the gpsimd
                    # queue, g_c1 (and the steps-0..7 projections) would
                    # block behind it.
                    add_dep_helper(g_c2.ins, g_c1.ins, sync=False,
                                   reason="queue order")

                # h_start = carry ? prev_block_tail : cinit
                hstart = hspool.tile([P, KT * BC], BF16, tag="hs")
                nc.vector.tensor_copy(hstart[:], cinit_sb[:, r])
                nc.vector.copy_predicated(
                    hstart[:], carry_sb[:, r], prev[:, :, T - 1, :]
                )

                def ship(h, t_lo, t_n):
                    """Send chunk h of cur to the successor's board."""
                    dd = nc.sync.dma_start(
                        hins[h][r].ap().rearrange("p (m t b) -> p m t b",
                                                  m=MT, t=t_n),
                        cur[:, :, t_lo:t_lo + t_n, :],
                    )
                    dxn = nc.sync.dma_start(
                        hands[h][r + 1][4 * P:5 * P, :],
                        x0t[r + 1][:, CH_OFF[h]:CH_OFF[h] + CHUNKS[h][2]])
                    cc = nc.gpsimd.collective_compute(
                        "AllGather",
                        mybir.AluOpType.bypass,
                        replica_groups=GROUPS,
                        ins=[hins[h][r][:]],
                        outs=[hands[h][r + 1][0:4 * P, :]],
                    )
                    add_dep_helper(cc.ins, dd.ins, sync=True,
                                   reason="AG after blk dma")
                    # keep the gpsimd queue in program order: an AG
                    # trigger floated ahead of a pending gather would
                    # head-of-line-block it on this trigger's DMA wait.
                    if g_c2 is not None:
                        add_dep_helper(cc.ins, g_c2.ins, sync=False,
                                       reason="queue order")
                    ccs[h] = cc
                    dx_prev[h] = dxn
                    return cc

                pp_live = None
                g_c0n = None
                for t in range(T):
                    last_mm = rec_step(cur, hstart, t)

                    # interleave projections into the tanh-latency gaps,
                    # half an m-pair k-sweep per step: steps 0..7 project
                    # this round's chunk 1 (t16..23), steps 8..15 chunk 2
                    # (t24..31), steps 24..31 the next round's chunk 0
                    # (gathered below after its mid-round AG).
                    if t < 8 and r > 0:
                        mp, kr = divmod(t, 2)
                        if kr == 0:
                            pp_live = pppool.tile([P, 2, TH, 16],
                                                  mybir.dt.float32, tag="pp")
                        proj_chunk(pp_live, xb, 1, mp, 4 * kr, 4 * kr + 4,
                                   after=last_mm)
                    if 8 <= t < 16 and r > 0:
                        mp, kr = divmod(t - 8, 2)
                        if kr == 0:
                            pp_live = pppool.tile([P, 2, TH, 16],
                                                  mybir.dt.float32, tag="pp")
                        proj_chunk(pp_live, xb, 2, mp, 4 * kr, 4 * kr + 4,
                                   after=last_mm)
                    if 24 <= t < 32 and r < ROUNDS - 1:
                        mp, kr = divmod(t - 24, 2)
                        if kr == 0:
                            pp_live = pppool.tile([P, 2, TH, 16],
                                                  mybir.dt.float32, tag="pp")
                        proj_chunk(pp_live, xb_next, 0, mp, 4 * kr, 4 * kr + 4,
                                   after=last_mm)

                    if r < ROUNDS - 1:
                        if t == TH - 1:
                            cc0 = ship(0, 0, TH)
                            g_c0n = gather(xb_next, 0, r + 1,
                                           [(cc0, "gather after AG"),
                                            (dx_prev[0], "gather after x-feed")])
                        elif t == TH + TQ - 1:
                            cc1 = ship(1, TH, TQ)
                            if g_c0n is not None:
                                add_dep_helper(cc1.ins, g_c0n.ins, sync=False,
                                               reason="queue order")

                # ship the last chunk at round end, BEFORE the fp32
                # output copy: the shared Sync DMA queue would otherwise
                # delay the AG launch behind the 1.2us cast.
                if r < ROUNDS - 1:
                    cc2 = ship(2, TH + TQ, TQ)
                    if g_c0n is not None:
                        add_dep_helper(cc2.ins, g_c0n.ins, sync=False,
                                       reason="queue order")

                # write fp32 output block
                o32 = opool.tile([P, MT * T * BC], F32, tag="o32")
                nc.vector.tensor_copy(o32[:], cur[:])
                nc.sync.dma_start(out[r], o32[:])
    nc.compile()
    return nc


def _prep_inputs(X, h0s, W, b):
    """Build the 8 per-core input maps."""
    in_maps = []
    for c in range(N_CORES):
        s, j = c // 2, c % 2
        Wl = np.asarray(W[s], dtype=np.float32)
        Wx, Wh = Wl[:, :D], Wl[:, D:]

        def tiles(M):  # M: [e, d] -> lhsT tiles [p, (k, m, q)]
            A = M.reshape(MT, P, KT, P)          # [m, q, k, p]
            return np.ascontiguousarray(
                A.transpose(3, 2, 0, 1).reshape(P, KT * MT * P)).astype(BF)

        whT = tiles(Wh)
        wxT = tiles(Wx)
        bias = np.ascontiguousarray(
            np.asarray(b[s], np.float32).reshape(MT, P).T)

        hin = np.asarray(h0s[s, BC * j:BC * (j + 1)], np.float32)  # [b, d]
        hinit = np.ascontiguousarray(
            hin.reshape(BC, KT, P).transpose(2, 1, 0).reshape(P, KT * BC)).astype(BF)

        carry = np.zeros((ROUNDS, P, KT * BC), np.uint8)
        cinit = np.zeros((ROUNDS, P, KT * BC), BF)
        for r in range(ROUNDS):
            if r > s:
                carry[r] = 1
            else:
                cinit[r] = hinit

        x0t = np.zeros((ROUNDS, P, BLK_COLS), BF)
        if s == 0:
            Xj = np.asarray(X[BC * j:BC * (j + 1)], np.float32)  # [b, L, d]
            # chunk-major: [(k,16t,b); (k,8t,b); (k,8t,b)]
            Xr = Xj.reshape(BC, NB, T, KT, P)  # [b, blk, t, k, p]
            parts = []
            for t_lo, t_n, _ in ((0, TH, 0), (TH, TQ, 0), (TH + TQ, TQ, 0)):
                c = Xr[:, :, t_lo:t_lo + t_n]  # [b, blk, tn, k, p]
                c = c.transpose(1, 4, 3, 2, 0)  # [blk, p, k, tn, b]
                parts.append(c.reshape(NB, P, KT * t_n * BC))
            Xb = np.ascontiguousarray(np.concatenate(parts, axis=2)).astype(BF)
            x0t[0:NB] = Xb
            gidx = (4 * P + np.arange(P, dtype=np.int32)).reshape(P, 1)
        else:  # stage s reads its predecessor (group position s-1)
            gidx = ((s - 1) * P + np.arange(P, dtype=np.int32)).reshape(P, 1)

        in_maps.append({
            "whT": whT, "wxT": wxT, "bias": bias,
            "carry": carry, "cinit": cinit,
            "gidx": gidx, "ident": np.eye(P, dtype=BF),
            "x0t": x0t,
        })
    return in_maps


def _extract(results):
    """Assemble full output [B, L, D] from stage-3 cores (6, 7)."""
    Y = np.empty((B, L, D), np.float32)
    for j in range(2):
        o = results[6 + j]["out"][NL - 1:NL - 1 + NB]   # [q, p, cols]
        o = o.reshape(NB, P, MT, T, BC).transpose(4, 0, 3, 2, 1)  # [b,q,t,m,p]
        Y[BC * j:BC * (j + 1)] = o.reshape(BC, L, D)
    return Y


def kernel(X, h0s, W, b, _trace=False):
    from concourse.bass_utils import run_bass_kernel_spmd

    if "nc" not in _cache:
        _cache["nc"] = _build()
    nc = _cache["nc"]
    in_maps = _prep_inputs(np.asarray(X), np.asarray(h0s), np.asarray(W),
                           np.asarray(b))
    res = run_bass_kernel_spmd(nc, in_maps, core_ids=list(range(N_CORES)),
                               trace=_trace)
    _cache["last_results"] = res
    return _extract(res.results)
